# revision 7
# baseline (speedup 1.0000x reference)
"""CrossAgentAttention Trainium2 kernel.

Problem: B=1024 samples, N=32 agents, D=512 features, H=4 heads (HD=128).
  qkv = x @ Win^T + bin ; per-head attention over the N=32 agents with the
  diagonal (self) and padded agents masked out of the keys; out = ctx @ Wout^T + bout.

Strategy (data-parallel over B across 8 cores, weights replicated):
  - Host pre-transposes the per-core activations to X^T [D, T] (T = B/8*32 tokens)
    and the weights to Win^T / Wout^T so every GEMM contraction dim lands on
    SBUF partitions.  Q columns of Win^T are pre-scaled by 1/sqrt(HD).
  - Stage 1: Q^T,K^T [1024, T] in transposed (feature-major) layout and
    V [T, 512] token-major, via f32r matmuls with N=512 moving operands.
  - Stage 2: attention per (sample-group of 4, head).  128 tokens = 4 samples
    x 32 agents: S = Q^T.T @ K^T gives all 16 cross-sample blocks; an additive
    mask kills cross-sample blocks, the self-diagonal, and padded keys.
    Softmax without max-subtraction (logits are O(1) by construction),
    exp+rowsum fused on ACT, P normalized on DVE, P^T via PE transpose,
    ctx^T = (V slice).T @ P^T.
  - Stage 3: OUT^T = Wout^T.T @ ctx^T, DMA out; host transposes back.
"""

import math

import numpy as np

import concourse.bass as bass
import concourse.mybir as mybir
import concourse.tile as tile
from concourse import bacc
from concourse.bass_utils import run_bass_kernel_spmd

N_CORES = 8
B, N, D, H = 1024, 32, 512, 4
HD = D // H  # 128
NEG = -60000.0  # additive mask value; exp() underflows to exactly 0.0
F32 = mybir.dt.float32
F32R = mybir.dt.float32r


def build_program(b_core, use_f32r=True, reps=1, with_pad=False, with_bias=False):
    """Trace + compile the per-core program. Returns (nc, meta)."""
    T = b_core * N  # tokens per core
    GT = 512 if T >= 512 else T  # tokens per group
    G = T // GT  # groups
    TT = GT // 128  # 128-token tiles (sample groups of 4) per group
    assert T % 512 == 0 or G == 1

    nc = bacc.Bacc("TRN2", target_bir_lowering=False, debug=False, num_devices=N_CORES)

    MD = F32R if use_f32r else F32  # matmul-operand dtype
    xt = nc.dram_tensor("xt", [D, T], MD, kind="ExternalInput").ap()
    wint = nc.dram_tensor("wint", [D, 3 * D], MD, kind="ExternalInput").ap()
    woutt = nc.dram_tensor("woutt", [D, D], MD, kind="ExternalInput").ap()
    ident = nc.dram_tensor("ident", [128, 128], F32, kind="ExternalInput").ap()
    if with_pad:
        mask = nc.dram_tensor("mask", [T // 128, 128, 4 * 128], F32,
                              kind="ExternalInput").ap()
    else:
        mask = nc.dram_tensor("mask", [128, 4 * 128], F32, kind="ExternalInput").ap()
    if with_bias:
        bqk = nc.dram_tensor("bqk", [128, 8], F32, kind="ExternalInput").ap()
        bv = nc.dram_tensor("bv", [1, D], MD, kind="ExternalInput").ap()
        bo = nc.dram_tensor("bo", [128, 4], F32, kind="ExternalInput").ap()
    outt = nc.dram_tensor("outt", [D, T], F32, kind="ExternalOutput").ap()

    with tile.TileContext(nc) as tc:
        with (
            tc.tile_pool(name="wpool", bufs=1) as wpool,
            tc.tile_pool(name="xtp", bufs=2 * 4, space="SBUF") as xtp,
            tc.tile_pool(name="qktp", bufs=2 * 8) as qktp,
            tc.tile_pool(name="vp", bufs=2 * TT) as vp,
            tc.tile_pool(name="smp", bufs=3) as smp,
            tc.tile_pool(name="ctxp", bufs=2) as ctxp,
            tc.tile_pool(name="otp", bufs=4) as otp,
            tc.tile_pool(name="mmps", bufs=3, space="PSUM") as mmps,
            tc.tile_pool(name="atps", bufs=5, space="PSUM") as atps,
        ):
            # ---- resident weights / constants ----
            w = []
            for k in range(4):
                wt = wpool.tile([128, 3 * D], MD, tag=f"wint{k}")
                nc.sync.dma_start(wt[:], wint[k * 128:(k + 1) * 128, :])
                w.append(wt)
            wo = []
            for k in range(4):
                wt = wpool.tile([128, D], MD, tag=f"woutt{k}")
                nc.sync.dma_start(wt[:], woutt[k * 128:(k + 1) * 128, :])
                wo.append(wt)
            idt = wpool.tile([128, 128], F32, tag="ident")
            nc.sync.dma_start(idt[:], ident[:])
            mk_const = None
            if not with_pad:
                mk_const = wpool.tile([128, 4 * 128], F32, tag="mask")
                nc.sync.dma_start(mk_const[:], mask[:])
            if with_bias:
                bqk_sb = wpool.tile([128, 8], F32, tag="bqk")
                nc.sync.dma_start(bqk_sb[:], bqk[:])
                bv_sb = wpool.tile([1, D], MD, tag="bv")
                nc.sync.dma_start(bv_sb[:], bv[:])
                bo_sb = wpool.tile([128, 4], F32, tag="bo")
                nc.sync.dma_start(bo_sb[:], bo[:])
                ones_sb = wpool.tile([1, 128], MD, tag="ones")
                nc.vector.memset(ones_sb[:], 1.0)

            def body(_iv=None):
                for g in range(G):
                    gsl = bass.ts(g, GT)
                    # ---- load X^T tiles for this group ----
                    xg = []
                    for k in range(4):
                        t = xtp.tile([128, GT], MD, tag="xt")
                        nc.sync.dma_start(t[:], xt[k * 128:(k + 1) * 128, gsl])
                        xg.append(t)
                    # ---- stage 1a: Q^T, K^T (feature-major) ----
                    qkt = []
                    for fo in range(8):
                        ps = mmps.tile([128, GT], F32, tag="mm")
                        for k in range(4):
                            nc.tensor.matmul(
                                ps[:],
                                w[k][:, bass.ts(fo, 128)],
                                xg[k][:],
                                start=(k == 0), stop=(k == 3),
                            )
                        qt = qktp.tile([128, GT], MD, tag="qkt")
                        if with_bias:
                            nc.scalar.activation(
                                qt[:], ps[:], mybir.ActivationFunctionType.Identity,
                                bias=bqk_sb[:, fo:fo + 1])
                        else:
                            nc.vector.tensor_copy(qt[:], ps[:])
                        qkt.append(qt)
                    # ---- stage 1b: V (token-major) ----
                    vg = []
                    for tt in range(TT):
                        ps = mmps.tile([128, D], F32, tag="mm")
                        for k in range(4):
                            nc.tensor.matmul(
                                ps[:],
                                xg[k][:, bass.ts(tt, 128)],
                                w[k][:, 2 * D:3 * D],
                                start=(k == 0), stop=(k == 3 and not with_bias),
                            )
                        if with_bias:
                            nc.tensor.matmul(
                                ps[:], ones_sb[:],
                                bv_sb[:],
                                start=False, stop=True,
                            )
                        vt = vp.tile([128, D], MD, tag="v")
                        nc.vector.tensor_copy(vt[:], ps[:])
                        vg.append(vt)
                    # ---- stage 2: attention per 128-token tile ----
                    ctxt = ctxp.tile([128, 4, GT], MD, tag="ctxt")
                    for tt in range(TT):
                        ttsl = bass.ts(tt, 128)
                        if with_pad:
                            mk = smp.tile([128, 4 * 128], F32, tag="mask")
                            nc.sync.dma_start(mk[:], mask[g * TT + tt])
                        else:
                            mk = mk_const
                        sps = atps.tile([128, 4 * 128], F32, tag="attn")
                        for h in range(4):
                            nc.tensor.matmul(
                                sps[:, bass.ts(h, 128)],
                                qkt[h][:, ttsl],
                                qkt[4 + h][:, ttsl],
                                start=True, stop=True,
                            )
                        ssb = smp.tile([128, 4 * 128], F32, tag="ssb")
                        nc.vector.tensor_add(ssb[:], sps[:], mk[:])
                        psb = smp.tile([128, 4 * 128], F32, tag="psb")
                        rsum = smp.tile([128, 8], F32, tag="rsum")
                        for h in range(4):
                            nc.scalar.activation(
                                psb[:, bass.ts(h, 128)], ssb[:, bass.ts(h, 128)],
                                mybir.ActivationFunctionType.Exp,
                                accum_out=rsum[:, h:h + 1],
                            )
                        nc.vector.reciprocal(rsum[:, 4:8], rsum[:, 0:4])
                        pnb = smp.tile([128, 4 * 128], F32, tag="pnb")
                        for h in range(4):
                            nc.vector.tensor_scalar_mul(
                                pnb[:, bass.ts(h, 128)], psb[:, bass.ts(h, 128)],
                                rsum[:, 4 + h:5 + h])
                        ptps = atps.tile([128, 4 * 128], F32, tag="attn")
                        for h in range(4):
                            nc.tensor.transpose(
                                ptps[:, bass.ts(h, 128)], pnb[:, bass.ts(h, 128)],
                                idt[:])
                        ptsb = smp.tile([128, 4 * 128], MD, tag="ptsb")
                        nc.vector.tensor_copy(ptsb[:], ptps[:])
                        cps = atps.tile([128, 4 * 128], F32, tag="attn")
                        for h in range(4):
                            nc.tensor.matmul(
                                cps[:, bass.ts(h, 128)],
                                vg[tt][:, bass.ts(h, 128)],
                                ptsb[:, bass.ts(h, 128)],
                                start=True, stop=True,
                            )
                        nc.vector.tensor_copy(
                            ctxt[:, :, ttsl],
                            cps[:].rearrange("p (h q) -> p h q", h=4))
                    # ---- stage 3: out projection ----
                    for fo in range(4):
                        ps = mmps.tile([128, GT], F32, tag="mm")
                        for k in range(4):
                            nc.tensor.matmul(
                                ps[:],
                                wo[k][:, bass.ts(fo, 128)],
                                ctxt[:, k, :],
                                start=(k == 0), stop=(k == 3),
                            )
                        ot = otp.tile([128, GT], F32, tag="ot")
                        if with_bias:
                            nc.scalar.activation(
                                ot[:], ps[:], mybir.ActivationFunctionType.Identity,
                                bias=bo_sb[:, fo:fo + 1])
                        else:
                            nc.vector.tensor_copy(ot[:], ps[:])
                        nc.sync.dma_start(outt[fo * 128:(fo + 1) * 128, gsl], ot[:])

            if reps == 1:
                body()
            else:
                with tc.For_i(0, reps, 1) as iv:
                    body(iv)

    nc.compile()
    return nc


def make_host_inputs(agent_hiddens, padding_mask, in_proj_weight, in_proj_bias,
                     out_proj_weight, out_proj_bias):
    """Shard + lay out host-side numpy arrays. Returns (in_maps, flags)."""
    x = np.asarray(agent_hiddens, dtype=np.float32)
    pad = np.asarray(padding_mask)
    win = np.asarray(in_proj_weight, dtype=np.float32)
    bin_ = np.asarray(in_proj_bias, dtype=np.float32)
    wout = np.asarray(out_proj_weight, dtype=np.float32)
    bout = np.asarray(out_proj_bias, dtype=np.float32)

    b = x.shape[0]
    b_core = b // N_CORES
    T = b_core * N
    scale = 1.0 / math.sqrt(HD)

    with_pad = bool(pad.any())
    with_bias = bool(bin_.any() or bout.any())

    wint = np.ascontiguousarray(win.T)
    wint[:, :D] *= scale
    woutt = np.ascontiguousarray(wout.T)
    identity = np.eye(128, dtype=np.float32)

    # 128-token block mask: tokens (s, i) x (s', j); mask cross-sample blocks
    # and the global diagonal (self-attention).
    p = np.arange(128)
    blockmask = np.where((p[:, None] // 32 != p[None, :] // 32)
                         | (p[:, None] == p[None, :]), NEG, 0.0).astype(np.float32)
    mask_tile = np.tile(blockmask, (1, 4))  # [128, 512], head-replicated

    in_maps = []
    for c in range(N_CORES):
        xc = x[c * b_core:(c + 1) * b_core].reshape(T, D)
        m = {
            "xt": np.ascontiguousarray(xc.T),
            "wint": wint,
            "woutt": woutt,
            "ident": identity,
        }
        if with_pad:
            padc = pad[c * b_core:(c + 1) * b_core]  # [b_core, N]
            n_tt = T // 128
            mt = np.empty((n_tt, 128, 512), dtype=np.float32)
            for t in range(n_tt):
                # 4 samples in this tile; key-padding kills columns
                pr = padc[t * 4:(t + 1) * 4].reshape(128)  # [(s', j)] order
                tilemask = blockmask + np.where(pr[None, :], NEG, 0.0)
                mt[t] = np.tile(tilemask, (1, 4))
            m["mask"] = mt
        else:
            m["mask"] = mask_tile
        if with_bias:
            bq = bin_[:D] * scale
            bk = bin_[D:2 * D]
            m["bqk"] = np.ascontiguousarray(
                np.concatenate([bq, bk]).reshape(8, 128).T)
            m["bv"] = bin_[2 * D:3 * D].reshape(1, D).copy()
            m["bo"] = np.ascontiguousarray(bout.reshape(4, 128).T)
        in_maps.append(m)
    return in_maps, dict(b_core=b_core, with_pad=with_pad, with_bias=with_bias)


def assemble_output(results, b_core):
    outs = []
    for c in range(N_CORES):
        ot = results[c]["outt"]  # [D, T]
        outs.append(ot.T.reshape(b_core, N, D))
    return np.ascontiguousarray(np.concatenate(outs, axis=0))


_NC_CACHE = {}


def _get_nc(key_args):
    key = tuple(sorted(key_args.items()))
    if key not in _NC_CACHE:
        _NC_CACHE[key] = build_program(**key_args)
    return _NC_CACHE[key]


USE_F32R = True


def kernel(agent_hiddens, padding_mask, in_proj_weight, in_proj_bias,
           out_proj_weight, out_proj_bias):
    in_maps, meta = make_host_inputs(
        agent_hiddens, padding_mask, in_proj_weight, in_proj_bias,
        out_proj_weight, out_proj_bias)
    nc = _get_nc(dict(b_core=meta["b_core"], use_f32r=USE_F32R, reps=1,
                      with_pad=meta["with_pad"], with_bias=meta["with_bias"]))
    res = run_bass_kernel_spmd(nc, in_maps, list(range(N_CORES)))
    return assemble_output(res.results, meta["b_core"])


# revision 11
# speedup vs baseline: 1.7354x; 1.7354x over previous
"""CrossAgentAttention Trainium2 kernel.

Problem: B=1024 samples, N=32 agents, D=512 features, H=4 heads (HD=128).
  qkv = x @ Win^T + bin ; per-head attention over the N=32 agents with the
  diagonal (self) and padded agents masked out of the keys; out = ctx @ Wout^T + bout.

Strategy (data-parallel over B across 8 cores, weights replicated):
  - Host pre-transposes the per-core activations to X^T [D, T] (T = B/8*32 tokens)
    and the weights to Win^T / Wout^T so every GEMM contraction dim lands on
    SBUF partitions.  Q columns of Win^T are pre-scaled by 1/sqrt(HD).
  - Stage 1: Q^T,K^T [1024, T] in transposed (feature-major) layout and
    V [T, 512] token-major, via f32r matmuls with N=512 moving operands.
  - Stage 2: attention per (sample-group of 4, head).  128 tokens = 4 samples
    x 32 agents: S = Q^T.T @ K^T gives all 16 cross-sample blocks; an additive
    mask kills cross-sample blocks, the self-diagonal, and padded keys.
    Softmax without max-subtraction (logits are O(1) by construction),
    exp+rowsum fused on ACT, P normalized on DVE, P^T via PE transpose,
    ctx^T = (V slice).T @ P^T.
  - Stage 3: OUT^T = Wout^T.T @ ctx^T, DMA out; host transposes back.
"""

import math

import numpy as np

import concourse.bass as bass
import concourse.mybir as mybir
import concourse.tile as tile
from concourse import bacc
from concourse.bass_utils import run_bass_kernel_spmd

N_CORES = 8
B, N, D, H = 1024, 32, 512, 4
HD = D // H  # 128
NEG = -60000.0  # additive mask value; exp() underflows to exactly 0.0
F32 = mybir.dt.float32
F32R = mybir.dt.float32r


def build_program(b_core, use_f32r=True, reps=1, with_pad=False, with_bias=False):
    """Trace + compile the per-core program. Returns (nc, meta)."""
    T = b_core * N  # tokens per core
    GT = 512 if T >= 512 else T  # tokens per group
    G = T // GT  # groups
    TT = GT // 128  # 128-token tiles (sample groups of 4) per group
    assert T % 512 == 0 or G == 1

    nc = bacc.Bacc("TRN2", target_bir_lowering=False, debug=False, num_devices=N_CORES)

    MD = F32R if use_f32r else F32  # matmul-operand dtype
    xt = nc.dram_tensor("xt", [D, T], MD, kind="ExternalInput").ap()
    wint = nc.dram_tensor("wint", [D, 3 * D], MD, kind="ExternalInput").ap()
    woutt = nc.dram_tensor("woutt", [D, D], MD, kind="ExternalInput").ap()
    ident = nc.dram_tensor("ident", [128, 128], MD, kind="ExternalInput").ap()
    if with_pad:
        mask = nc.dram_tensor("mask", [T // 128, 128, 4 * 128], F32,
                              kind="ExternalInput").ap()
    else:
        mask = nc.dram_tensor("mask", [128, 4 * 128], F32, kind="ExternalInput").ap()
    if with_bias:
        bqk = nc.dram_tensor("bqk", [128, 8], F32, kind="ExternalInput").ap()
        bv = nc.dram_tensor("bv", [1, D], MD, kind="ExternalInput").ap()
        bo = nc.dram_tensor("bo", [128, 4], F32, kind="ExternalInput").ap()
    outt = nc.dram_tensor("outt", [D, T], F32, kind="ExternalOutput").ap()

    with tile.TileContext(nc) as tc:
        with (
            tc.tile_pool(name="wpool", bufs=1) as wpool,
            tc.tile_pool(name="xtp", bufs=2 * 4, space="SBUF") as xtp,
            tc.tile_pool(name="qktp", bufs=2 * 8) as qktp,
            tc.tile_pool(name="vp", bufs=2 * TT) as vp,
            tc.tile_pool(name="smp", bufs=4) as smp,
            tc.tile_pool(name="ctxp", bufs=2) as ctxp,
            tc.tile_pool(name="otp", bufs=4) as otp,
            tc.tile_pool(name="mmps", bufs=2, space="PSUM") as mmps,
            tc.tile_pool(name="atps", bufs=6, space="PSUM") as atps,
        ):
            # ---- resident weights / constants ----
            w = []
            for k in range(4):
                wt = wpool.tile([128, 3 * D], MD, tag=f"wint{k}")
                nc.sync.dma_start(wt[:], wint[k * 128:(k + 1) * 128, :])
                w.append(wt)
            wo = []
            for k in range(4):
                wt = wpool.tile([128, D], MD, tag=f"woutt{k}")
                nc.sync.dma_start(wt[:], woutt[k * 128:(k + 1) * 128, :])
                wo.append(wt)
            idt = wpool.tile([128, 128], MD, tag="ident")
            nc.sync.dma_start(idt[:], ident[:])
            mk_const = None
            if not with_pad:
                mk_const = wpool.tile([128, 4 * 128], F32, tag="mask")
                nc.sync.dma_start(mk_const[:], mask[:])
            if with_bias:
                bqk_sb = wpool.tile([128, 8], F32, tag="bqk")
                nc.sync.dma_start(bqk_sb[:], bqk[:])
                bv_sb = wpool.tile([1, D], MD, tag="bv")
                nc.sync.dma_start(bv_sb[:], bv[:])
                bo_sb = wpool.tile([128, 4], F32, tag="bo")
                nc.sync.dma_start(bo_sb[:], bo[:])
                ones_sb = wpool.tile([1, 128], MD, tag="ones")
                nc.vector.memset(ones_sb[:], 1.0)

            def body(_iv=None):
                def emit_outproj(ctxt_prev, gsl_prev):
                    for fo in range(4):
                        ps = mmps.tile([128, GT], F32, tag="mm")
                        for k in range(4):
                            nc.tensor.matmul(
                                ps[:],
                                wo[k][:, bass.ts(fo, 128)],
                                ctxt_prev[:, k, :],
                                start=(k == 0), stop=(k == 3),
                            )
                        ot = otp.tile([128, GT], F32, tag="ot")
                        if with_bias:
                            nc.scalar.activation(
                                ot[:], ps[:], mybir.ActivationFunctionType.Identity,
                                bias=bo_sb[:, fo:fo + 1])
                        else:
                            nc.scalar.copy(ot[:], ps[:])
                        nc.sync.dma_start(
                            outt[fo * 128:(fo + 1) * 128, gsl_prev], ot[:])

                pending = None
                for g in range(G):
                    gsl = bass.ts(g, GT)
                    # ---- load X^T tiles for this group (ACT HWDGE ring) ----
                    xg = []
                    for k in range(4):
                        t = xtp.tile([128, GT], MD, tag="xt")
                        nc.scalar.dma_start(t[:], xt[k * 128:(k + 1) * 128, gsl])
                        xg.append(t)
                    # ---- stage 1a: Q^T, K^T (feature-major) ----
                    qkt = []
                    for fo in range(8):
                        ps = mmps.tile([128, GT], F32, tag="mm")
                        for k in range(4):
                            nc.tensor.matmul(
                                ps[:],
                                w[k][:, bass.ts(fo, 128)],
                                xg[k][:],
                                start=(k == 0), stop=(k == 3),
                            )
                        qt = qktp.tile([128, GT], MD, tag="qkt")
                        if with_bias:
                            nc.scalar.activation(
                                qt[:], ps[:], mybir.ActivationFunctionType.Identity,
                                bias=bqk_sb[:, fo:fo + 1])
                        else:
                            nc.vector.tensor_copy(qt[:], ps[:])
                        qkt.append(qt)
                    # ---- stage 1b: V (token-major) ----
                    vg = []
                    for tt in range(TT):
                        ps = mmps.tile([128, D], F32, tag="mm")
                        for k in range(4):
                            nc.tensor.matmul(
                                ps[:],
                                xg[k][:, bass.ts(tt, 128)],
                                w[k][:, 2 * D:3 * D],
                                start=(k == 0), stop=(k == 3 and not with_bias),
                            )
                        if with_bias:
                            nc.tensor.matmul(
                                ps[:], ones_sb[:],
                                bv_sb[:],
                                start=False, stop=True,
                            )
                        vt = vp.tile([128, D], MD, tag="v")
                        nc.scalar.copy(vt[:], ps[:])
                        vg.append(vt)
                    # ---- out-proj of the previous group fills PE here ----
                    if pending is not None:
                        emit_outproj(*pending)
                    # ---- stage 2: attention, software-pipelined over tt ----
                    ctxt = ctxp.tile([128, 4, GT], MD, tag="ctxt")
                    pnbs, ptsbs = {}, {}

                    def stA(tt):
                        ttsl = bass.ts(tt, 128)
                        if with_pad:
                            mk = smp.tile([128, 4 * 128], F32, tag="mask")
                            nc.sync.dma_start(mk[:], mask[g * TT + tt])
                        else:
                            mk = mk_const
                        sps = atps.tile([128, 4 * 128], F32, tag="attn")
                        for h in range(4):
                            nc.tensor.matmul(
                                sps[:, bass.ts(h, 128)],
                                qkt[h][:, ttsl],
                                qkt[4 + h][:, ttsl],
                                start=True, stop=True,
                            )
                        ssb = smp.tile([128, 4 * 128], F32, tag="ssb")
                        nc.vector.tensor_add(ssb[:], sps[:], mk[:])
                        psb = smp.tile([128, 4 * 128], F32, tag="psb")
                        rsum = smp.tile([128, 8], F32, tag="rsum")
                        nc.scalar.activation(
                            psb[:], ssb[:], mybir.ActivationFunctionType.Exp)
                        nc.vector.reduce_sum(
                            rsum[:, 0:4],
                            psb[:].rearrange("p (h j) -> p h j", h=4),
                            axis=mybir.AxisListType.X)
                        nc.vector.reciprocal(rsum[:, 4:8], rsum[:, 0:4])
                        pnb = smp.tile([128, 4 * 128], MD, tag="pnb")
                        rb = rsum[:, 4:8]
                        rinv_b = bass.AP(tensor=rb.tensor, offset=rb.offset,
                                         ap=list(rb.ap) + [[0, 128]])
                        nc.vector.tensor_mul(
                            pnb[:].rearrange("p (h j) -> p h j", h=4),
                            psb[:].rearrange("p (h j) -> p h j", h=4),
                            rinv_b)
                        pnbs[tt] = pnb

                    def stB(tt):
                        pnb = pnbs.pop(tt)
                        ptps = atps.tile([128, 4 * 128], MD, tag="attn")
                        for h in range(4):
                            nc.tensor.transpose(
                                ptps[:, bass.ts(h, 128)], pnb[:, bass.ts(h, 128)],
                                idt[:])
                        ptsb = smp.tile([128, 4 * 128], MD, tag="ptsb")
                        nc.scalar.copy(ptsb[:], ptps[:])
                        ptsbs[tt] = ptsb

                    def stC(tt):
                        ttsl = bass.ts(tt, 128)
                        ptsb = ptsbs.pop(tt)
                        cps = atps.tile([128, 4 * 128], F32, tag="attn")
                        for h in range(4):
                            nc.tensor.matmul(
                                cps[:, bass.ts(h, 128)],
                                vg[tt][:, bass.ts(h, 128)],
                                ptsb[:, bass.ts(h, 128)],
                                start=True, stop=True,
                            )
                        nc.vector.tensor_copy(
                            ctxt[:, :, ttsl],
                            cps[:].rearrange("p (h q) -> p h q", h=4))

                    lag = min(2, TT - 1)
                    for t in range(TT + lag):
                        if t < TT:
                            stA(t)
                        if t >= lag:
                            stB(t - lag)
                            stC(t - lag)
                    pending = (ctxt, gsl)
                emit_outproj(*pending)

            if reps == 1:
                body()
            else:
                with tc.For_i(0, reps, 1, hint_engines=(
                        mybir.EngineType.PE, mybir.EngineType.DVE,
                        mybir.EngineType.Activation, mybir.EngineType.SP)) as iv:
                    body(iv)

    nc.compile()
    return nc


def make_host_inputs(agent_hiddens, padding_mask, in_proj_weight, in_proj_bias,
                     out_proj_weight, out_proj_bias):
    """Shard + lay out host-side numpy arrays. Returns (in_maps, flags)."""
    x = np.asarray(agent_hiddens, dtype=np.float32)
    pad = np.asarray(padding_mask)
    win = np.asarray(in_proj_weight, dtype=np.float32)
    bin_ = np.asarray(in_proj_bias, dtype=np.float32)
    wout = np.asarray(out_proj_weight, dtype=np.float32)
    bout = np.asarray(out_proj_bias, dtype=np.float32)

    b = x.shape[0]
    b_core = b // N_CORES
    T = b_core * N
    scale = 1.0 / math.sqrt(HD)

    with_pad = bool(pad.any())
    with_bias = bool(bin_.any() or bout.any())

    wint = np.ascontiguousarray(win.T)
    wint[:, :D] *= scale
    woutt = np.ascontiguousarray(wout.T)
    identity = np.eye(128, dtype=np.float32)

    # 128-token block mask: tokens (s, i) x (s', j); mask cross-sample blocks
    # and the global diagonal (self-attention).
    p = np.arange(128)
    blockmask = np.where((p[:, None] // 32 != p[None, :] // 32)
                         | (p[:, None] == p[None, :]), NEG, 0.0).astype(np.float32)
    mask_tile = np.tile(blockmask, (1, 4))  # [128, 512], head-replicated

    in_maps = []
    for c in range(N_CORES):
        xc = x[c * b_core:(c + 1) * b_core].reshape(T, D)
        m = {
            "xt": np.ascontiguousarray(xc.T),
            "wint": wint,
            "woutt": woutt,
            "ident": identity,
        }
        if with_pad:
            padc = pad[c * b_core:(c + 1) * b_core]  # [b_core, N]
            n_tt = T // 128
            mt = np.empty((n_tt, 128, 512), dtype=np.float32)
            for t in range(n_tt):
                # 4 samples in this tile; key-padding kills columns
                pr = padc[t * 4:(t + 1) * 4].reshape(128)  # [(s', j)] order
                tilemask = blockmask + np.where(pr[None, :], NEG, 0.0)
                mt[t] = np.tile(tilemask, (1, 4))
            m["mask"] = mt
        else:
            m["mask"] = mask_tile
        if with_bias:
            bq = bin_[:D] * scale
            bk = bin_[D:2 * D]
            m["bqk"] = np.ascontiguousarray(
                np.concatenate([bq, bk]).reshape(8, 128).T)
            m["bv"] = bin_[2 * D:3 * D].reshape(1, D).copy()
            m["bo"] = np.ascontiguousarray(bout.reshape(4, 128).T)
        in_maps.append(m)
    return in_maps, dict(b_core=b_core, with_pad=with_pad, with_bias=with_bias)


def assemble_output(results, b_core):
    outs = []
    for c in range(N_CORES):
        ot = results[c]["outt"]  # [D, T]
        outs.append(ot.T.reshape(b_core, N, D))
    return np.ascontiguousarray(np.concatenate(outs, axis=0))


_NC_CACHE = {}


def _get_nc(key_args):
    key = tuple(sorted(key_args.items()))
    if key not in _NC_CACHE:
        _NC_CACHE[key] = build_program(**key_args)
    return _NC_CACHE[key]


USE_F32R = True


def kernel(agent_hiddens, padding_mask, in_proj_weight, in_proj_bias,
           out_proj_weight, out_proj_bias):
    in_maps, meta = make_host_inputs(
        agent_hiddens, padding_mask, in_proj_weight, in_proj_bias,
        out_proj_weight, out_proj_bias)
    nc = _get_nc(dict(b_core=meta["b_core"], use_f32r=USE_F32R, reps=1,
                      with_pad=meta["with_pad"], with_bias=meta["with_bias"]))
    res = run_bass_kernel_spmd(nc, in_maps, list(range(N_CORES)))
    return assemble_output(res.results, meta["b_core"])


# revision 15
# speedup vs baseline: 1.8226x; 1.0503x over previous
"""CrossAgentAttention Trainium2 kernel.

Problem: B=1024 samples, N=32 agents, D=512 features, H=4 heads (HD=128).
  qkv = x @ Win^T + bin ; per-head attention over the N=32 agents with the
  diagonal (self) and padded agents masked out of the keys; out = ctx @ Wout^T + bout.

Strategy (data-parallel over B across 8 cores, weights replicated):
  - Host pre-transposes the per-core activations to X^T [D, T] (T = B/8*32 tokens)
    and the weights to Win^T / Wout^T so every GEMM contraction dim lands on
    SBUF partitions.  Q columns of Win^T are pre-scaled by 1/sqrt(HD).
  - Stage 1: Q^T,K^T [1024, T] in transposed (feature-major) layout and
    V [T, 512] token-major, via f32r matmuls with N=512 moving operands.
  - Stage 2: attention per (sample-group of 4, head).  128 tokens = 4 samples
    x 32 agents: S = Q^T.T @ K^T gives all 16 cross-sample blocks; an additive
    mask kills cross-sample blocks, the self-diagonal, and padded keys.
    Softmax without max-subtraction (logits are O(1) by construction),
    exp+rowsum fused on ACT, P normalized on DVE, P^T via PE transpose,
    ctx^T = (V slice).T @ P^T.
  - Stage 3: OUT^T = Wout^T.T @ ctx^T, DMA out; host transposes back.
"""

import math

import numpy as np

import concourse.bass as bass
import concourse.mybir as mybir
import concourse.tile as tile
from concourse import bacc
from concourse.bass_utils import run_bass_kernel_spmd

N_CORES = 8
B, N, D, H = 1024, 32, 512, 4
HD = D // H  # 128
NEG = -60000.0  # additive mask value; exp() underflows to exactly 0.0
F32 = mybir.dt.float32
F32R = mybir.dt.float32r


def build_program(b_core, use_f32r=True, reps=1, with_pad=False, with_bias=False):
    """Trace + compile the per-core program. Returns (nc, meta)."""
    T = b_core * N  # tokens per core
    GT = 512 if T >= 512 else T  # tokens per group
    G = T // GT  # groups
    TT = GT // 128  # 128-token tiles (sample groups of 4) per group
    assert T % 512 == 0 or G == 1

    nc = bacc.Bacc("TRN2", target_bir_lowering=False, debug=False, num_devices=N_CORES)

    MD = F32R if use_f32r else F32  # matmul-operand dtype
    xt = nc.dram_tensor("xt", [D, T], MD, kind="ExternalInput").ap()
    wint = nc.dram_tensor("wint", [D, 3 * D], MD, kind="ExternalInput").ap()
    woutt = nc.dram_tensor("woutt", [D, D], MD, kind="ExternalInput").ap()
    ident = nc.dram_tensor("ident", [128, 128], MD, kind="ExternalInput").ap()
    if with_pad:
        mask = nc.dram_tensor("mask", [T // 128, 128, 4 * 128], F32,
                              kind="ExternalInput").ap()
    else:
        mask = nc.dram_tensor("mask", [128, 4 * 128], F32, kind="ExternalInput").ap()
    if with_bias:
        bqk = nc.dram_tensor("bqk", [128, 8], F32, kind="ExternalInput").ap()
        bv = nc.dram_tensor("bv", [1, D], MD, kind="ExternalInput").ap()
        bo = nc.dram_tensor("bo", [128, 4], F32, kind="ExternalInput").ap()
    outt = nc.dram_tensor("outt", [D, T], F32, kind="ExternalOutput").ap()

    with tile.TileContext(nc) as tc:
        with (
            tc.tile_pool(name="wpool", bufs=1) as wpool,
            tc.tile_pool(name="xtp", bufs=2 * 4, space="SBUF") as xtp,
            tc.tile_pool(name="qktp", bufs=2 * 8) as qktp,
            tc.tile_pool(name="vp", bufs=2 * TT) as vp,
            tc.tile_pool(name="smp", bufs=4) as smp,
            tc.tile_pool(name="ctxp", bufs=2) as ctxp,
            tc.tile_pool(name="otp", bufs=4) as otp,
            tc.tile_pool(name="mmps", bufs=2, space="PSUM") as mmps,
            tc.tile_pool(name="atps", bufs=6, space="PSUM") as atps,
        ):
            # ---- resident weights / constants ----
            w = []
            for k in range(4):
                wt = wpool.tile([128, 3 * D], MD, tag=f"wint{k}")
                w.append(wt)
            # chunked so Q columns (chunk 0) land first; K then V follow
            for c in range(3):
                for k in range(4):
                    nc.sync.dma_start(
                        w[k][:, bass.ts(c, D)],
                        wint[k * 128:(k + 1) * 128, bass.ts(c, D)])
            idt = wpool.tile([128, 128], MD, tag="ident")
            nc.sync.dma_start(idt[:], ident[:])
            mk_const = None
            if not with_pad:
                mk_const = wpool.tile([128, 4 * 128], F32, tag="mask")
                nc.sync.dma_start(mk_const[:], mask[:])
            wo = []
            for k in range(4):
                wt = wpool.tile([128, D], MD, tag=f"woutt{k}")
                nc.sync.dma_start(wt[:], woutt[k * 128:(k + 1) * 128, :])
                wo.append(wt)
            if with_bias:
                bqk_sb = wpool.tile([128, 8], F32, tag="bqk")
                nc.sync.dma_start(bqk_sb[:], bqk[:])
                bv_sb = wpool.tile([1, D], MD, tag="bv")
                nc.sync.dma_start(bv_sb[:], bv[:])
                bo_sb = wpool.tile([128, 4], F32, tag="bo")
                nc.sync.dma_start(bo_sb[:], bo[:])
                ones_sb = wpool.tile([1, 128], MD, tag="ones")
                nc.vector.memset(ones_sb[:], 1.0)

            def body(_iv=None):
                def emit_outproj(ctxt_prev, gsl_prev):
                    for fo in range(4):
                        ps = mmps.tile([128, GT], F32, tag="mm")
                        for k in range(4):
                            nc.tensor.matmul(
                                ps[:],
                                wo[k][:, bass.ts(fo, 128)],
                                ctxt_prev[:, k, :],
                                start=(k == 0), stop=(k == 3),
                            )
                        ot = otp.tile([128, GT], F32, tag="ot")
                        if with_bias:
                            nc.scalar.activation(
                                ot[:], ps[:], mybir.ActivationFunctionType.Identity,
                                bias=bo_sb[:, fo:fo + 1])
                        else:
                            nc.scalar.copy(ot[:], ps[:])
                        nc.sync.dma_start(
                            outt[fo * 128:(fo + 1) * 128, gsl_prev], ot[:])

                pending = None
                for g in range(G):
                    gsl = bass.ts(g, GT)
                    # ---- load X^T tiles for this group (ACT HWDGE ring) ----
                    xg = []
                    for k in range(4):
                        t = xtp.tile([128, GT], MD, tag="xt")
                        nc.scalar.dma_start(t[:], xt[k * 128:(k + 1) * 128, gsl])
                        xg.append(t)
                    # ---- stage 1a: Q^T, K^T (feature-major) ----
                    qkt = []
                    for fo in range(8):
                        ps = mmps.tile([128, GT], F32, tag="mm")
                        for k in range(4):
                            nc.tensor.matmul(
                                ps[:],
                                w[k][:, bass.ts(fo, 128)],
                                xg[k][:],
                                start=(k == 0), stop=(k == 3),
                            )
                        qt = qktp.tile([128, GT], MD, tag="qkt")
                        if with_bias:
                            nc.scalar.activation(
                                qt[:], ps[:], mybir.ActivationFunctionType.Identity,
                                bias=bqk_sb[:, fo:fo + 1])
                        else:
                            nc.vector.tensor_copy(qt[:], ps[:])
                        qkt.append(qt)
                    # ---- stage 1b: V (token-major) ----
                    vg = []
                    for tt in range(TT):
                        ps = mmps.tile([128, D], F32, tag="mm")
                        for k in range(4):
                            nc.tensor.matmul(
                                ps[:],
                                xg[k][:, bass.ts(tt, 128)],
                                w[k][:, 2 * D:3 * D],
                                start=(k == 0), stop=(k == 3 and not with_bias),
                            )
                        if with_bias:
                            nc.tensor.matmul(
                                ps[:], ones_sb[:],
                                bv_sb[:],
                                start=False, stop=True,
                            )
                        vt = vp.tile([128, D], MD, tag="v")
                        nc.scalar.copy(vt[:], ps[:])
                        vg.append(vt)
                    # ---- out-proj of the previous group fills PE here ----
                    if pending is not None:
                        emit_outproj(*pending)
                    # ---- stage 2: attention, software-pipelined over tt ----
                    ctxt = ctxp.tile([128, 4, GT], MD, tag="ctxt")
                    pnbs, ptsbs = {}, {}

                    def stA(tt):
                        ttsl = bass.ts(tt, 128)
                        if with_pad:
                            mk = smp.tile([128, 4 * 128], F32, tag="mask")
                            nc.sync.dma_start(mk[:], mask[g * TT + tt])
                        else:
                            mk = mk_const
                        sps = atps.tile([128, 4 * 128], F32, tag="attn")
                        for h in range(4):
                            nc.tensor.matmul(
                                sps[:, bass.ts(h, 128)],
                                qkt[h][:, ttsl],
                                qkt[4 + h][:, ttsl],
                                start=True, stop=True,
                            )
                        ssb = smp.tile([128, 4 * 128], F32, tag="ssb")
                        nc.vector.tensor_add(ssb[:], sps[:], mk[:])
                        psb = smp.tile([128, 4 * 128], F32, tag="psb")
                        rsum = smp.tile([128, 8], F32, tag="rsum")
                        nc.scalar.activation(
                            psb[:], ssb[:], mybir.ActivationFunctionType.Exp)
                        nc.vector.reduce_sum(
                            rsum[:, 0:4],
                            psb[:].rearrange("p (h j) -> p h j", h=4),
                            axis=mybir.AxisListType.X)
                        nc.vector.reciprocal(rsum[:, 4:8], rsum[:, 0:4])
                        pnb = smp.tile([128, 4 * 128], MD, tag="pnb")
                        rb = rsum[:, 4:8]
                        rinv_b = bass.AP(tensor=rb.tensor, offset=rb.offset,
                                         ap=list(rb.ap) + [[0, 128]])
                        nc.vector.tensor_mul(
                            pnb[:].rearrange("p (h j) -> p h j", h=4),
                            psb[:].rearrange("p (h j) -> p h j", h=4),
                            rinv_b)
                        pnbs[tt] = pnb

                    def stB(tt):
                        pnb = pnbs.pop(tt)
                        ptps = atps.tile([128, 4 * 128], MD, tag="attn")
                        for h in range(4):
                            nc.tensor.transpose(
                                ptps[:, bass.ts(h, 128)], pnb[:, bass.ts(h, 128)],
                                idt[:])
                        ptsb = smp.tile([128, 4 * 128], MD, tag="ptsb")
                        nc.scalar.copy(ptsb[:], ptps[:])
                        ptsbs[tt] = ptsb

                    def stC(tt):
                        ttsl = bass.ts(tt, 128)
                        ptsb = ptsbs.pop(tt)
                        cps = atps.tile([128, 4 * 128], F32, tag="attn")
                        for h in range(4):
                            nc.tensor.matmul(
                                cps[:, bass.ts(h, 128)],
                                vg[tt][:, bass.ts(h, 128)],
                                ptsb[:, bass.ts(h, 128)],
                                start=True, stop=True,
                            )
                        nc.scalar.copy(
                            ctxt[:, :, ttsl],
                            cps[:].rearrange("p (h q) -> p h q", h=4))

                    lag = min(3, TT - 1)
                    for t in range(TT + lag):
                        if t < TT:
                            stA(t)
                        if t >= lag:
                            stB(t - lag)
                            stC(t - lag)
                    pending = (ctxt, gsl)
                emit_outproj(*pending)

            if reps == 1:
                body()
            else:
                with tc.For_i(0, reps, 1, hint_engines=(
                        mybir.EngineType.PE, mybir.EngineType.DVE,
                        mybir.EngineType.Activation, mybir.EngineType.SP)) as iv:
                    body(iv)

    nc.compile()
    return nc


def make_host_inputs(agent_hiddens, padding_mask, in_proj_weight, in_proj_bias,
                     out_proj_weight, out_proj_bias):
    """Shard + lay out host-side numpy arrays. Returns (in_maps, flags)."""
    x = np.asarray(agent_hiddens, dtype=np.float32)
    pad = np.asarray(padding_mask)
    win = np.asarray(in_proj_weight, dtype=np.float32)
    bin_ = np.asarray(in_proj_bias, dtype=np.float32)
    wout = np.asarray(out_proj_weight, dtype=np.float32)
    bout = np.asarray(out_proj_bias, dtype=np.float32)

    b = x.shape[0]
    b_core = b // N_CORES
    T = b_core * N
    scale = 1.0 / math.sqrt(HD)

    with_pad = bool(pad.any())
    with_bias = bool(bin_.any() or bout.any())

    wint = np.ascontiguousarray(win.T)
    wint[:, :D] *= scale
    woutt = np.ascontiguousarray(wout.T)
    identity = np.eye(128, dtype=np.float32)

    # 128-token block mask: tokens (s, i) x (s', j); mask cross-sample blocks
    # and the global diagonal (self-attention).
    p = np.arange(128)
    blockmask = np.where((p[:, None] // 32 != p[None, :] // 32)
                         | (p[:, None] == p[None, :]), NEG, 0.0).astype(np.float32)
    mask_tile = np.tile(blockmask, (1, 4))  # [128, 512], head-replicated

    in_maps = []
    for c in range(N_CORES):
        xc = x[c * b_core:(c + 1) * b_core].reshape(T, D)
        m = {
            "xt": np.ascontiguousarray(xc.T),
            "wint": wint,
            "woutt": woutt,
            "ident": identity,
        }
        if with_pad:
            padc = pad[c * b_core:(c + 1) * b_core]  # [b_core, N]
            n_tt = T // 128
            mt = np.empty((n_tt, 128, 512), dtype=np.float32)
            for t in range(n_tt):
                # 4 samples in this tile; key-padding kills columns
                pr = padc[t * 4:(t + 1) * 4].reshape(128)  # [(s', j)] order
                tilemask = blockmask + np.where(pr[None, :], NEG, 0.0)
                mt[t] = np.tile(tilemask, (1, 4))
            m["mask"] = mt
        else:
            m["mask"] = mask_tile
        if with_bias:
            bq = bin_[:D] * scale
            bk = bin_[D:2 * D]
            m["bqk"] = np.ascontiguousarray(
                np.concatenate([bq, bk]).reshape(8, 128).T)
            m["bv"] = bin_[2 * D:3 * D].reshape(1, D).copy()
            m["bo"] = np.ascontiguousarray(bout.reshape(4, 128).T)
        in_maps.append(m)
    return in_maps, dict(b_core=b_core, with_pad=with_pad, with_bias=with_bias)


def assemble_output(results, b_core):
    outs = []
    for c in range(N_CORES):
        ot = results[c]["outt"]  # [D, T]
        outs.append(ot.T.reshape(b_core, N, D))
    return np.ascontiguousarray(np.concatenate(outs, axis=0))


_NC_CACHE = {}


def _get_nc(key_args):
    key = tuple(sorted(key_args.items()))
    if key not in _NC_CACHE:
        _NC_CACHE[key] = build_program(**key_args)
    return _NC_CACHE[key]


USE_F32R = True


def kernel(agent_hiddens, padding_mask, in_proj_weight, in_proj_bias,
           out_proj_weight, out_proj_bias):
    in_maps, meta = make_host_inputs(
        agent_hiddens, padding_mask, in_proj_weight, in_proj_bias,
        out_proj_weight, out_proj_bias)
    nc = _get_nc(dict(b_core=meta["b_core"], use_f32r=USE_F32R, reps=1,
                      with_pad=meta["with_pad"], with_bias=meta["with_bias"]))
    res = run_bass_kernel_spmd(nc, in_maps, list(range(N_CORES)))
    return assemble_output(res.results, meta["b_core"])
